# revision 1
# baseline (speedup 1.0000x reference)
"""Bass/Trainium2 kernel for nn_Attention_6983616824195.

Single-head attention with Dense projections:
    q = Q @ WQ ; k = K @ WK ; v = V @ WV        (B, L, 128)
    S = q @ k^T ; S = where(mask==1, S, -inf) ; S /= sqrt(128)
    out = softmax(S, axis=-1) @ v               (B, L, 128)

Shapes: B=4, L=4096, DM=1024, DK=DV=128, mask [B, 1, L] (key mask).

Sharding: 8 cores = (batch b, KEY-half h). Core c = (b=c//2, h=c%2)
computes ALL queries of batch b against keys [h*2048, (h+1)*2048).
Key-sharding (vs. query-sharding) halves the K and V projections per
core and duplicates only the single Q projection — strictly less
duplicated matmul work, and less DMA. Each core returns the
unnormalized softmax numerator plus denominator for its key half
(exact partial sums — no running-max needed since the scaled logits
are ~N(0,1) and exp cannot overflow); the host adds the two halves
and divides. WQ/WK/WV are replicated.

Per-core dataflow (all matmuls contract over the SBUF partition dim):
  - Host supplies Q/K/V in a dm-blocked transposed fp16 layout so every
    DMA is a single instruction whose per-partition segments are 2-8KB
    contiguous, and no on-chip transposes are needed anywhere.
  - One fully-pipelined loop over the core's 4 key-blocks (512 keys
    each) overlaps K/V DMA, K/V projections, scores, exp, and AV.
  Per key-block sb:
    kT[d, s]    = sum_c WK[c]^T·KTB[c]     (lhsT=WK chunk, rhs=KT chunk)
    v[s, dv]    = sum_c VTB[c]^T·WV[c]     (lhsT=VT tile, rhs=WV chunk)
    vext[s, 0:128] = v*mask[s]; vext[s,128] = mask[s]   (ones column)
    (sb==0 only, per q-block: qT[d, q] = sum_c WQ[c]^T·QTB[c])
    S^T[s, q]   = kT^T·qT       (lhsT=kT s-tile, rhs=qT q-block; two
                                 s-tiles paired into one [128,1024] psum)
    e = exp(S^T / sqrt(128))    (one ScalarE op per pair, fp16 out)
    A[q, 0:129] += sum_s e^T·vext  (psum over the block's 4 s-tiles,
                                    then DVE-accumulated into SBUF f32;
                                    column 128 = denominator partial)
  Output O[q, 0:129] = A (numerator cols 0:128, denominator col 128).
Masking is exact: masked keys get weight mask[s]=0 in both numerator
and denominator, identical to where(mask==1, S, -inf) softmax.
"""

import numpy as np
import ml_dtypes

import jax

try:  # persistent compile cache: repeat calls skip the walrus compile
    jax.config.update("jax_compilation_cache_dir", "/tmp/jaxcache")
    jax.config.update("jax_persistent_cache_min_compile_time_secs", 1.0)
    jax.config.update("jax_persistent_cache_min_entry_size_bytes", 0)
except Exception:
    pass

import concourse.bass as bass
import concourse.tile as tile
import concourse.mybir as mybir
from concourse.bass_utils import run_bass_kernel_spmd

B, L, DM = 4, 4096, 1024
DK = DV = 128
N_CORES = 8
LQ = L                 # queries per core (all 4096 of the batch)
LK = L // 2            # keys per core (2048)
P = 128
NDC = DM // P          # dm chunks (8)
NQB = LQ // 512        # q blocks of 512 (8)
NQT_PER_B = 512 // P   # q tiles per block (4)
NST = LK // P          # s tiles per core (16)
NSB = LK // 512        # key blocks per core (4)
JPB = NST // NSB       # s tiles per key block (4)
VW = DV + 1            # v-ext width (129): 128 dv cols + ones column
SCALE = 1.0 / float(np.sqrt(DK))

F32 = mybir.dt.float32
F16 = mybir.dt.float16


def _split_multi_waits(nc, max_waits=1):
    """This walrus build encodes at most one sync-wait per instruction;
    move surplus waits onto preceding NoOps on the same engine."""
    for f in nc.m.functions:
        for bb in f.blocks:
            new_insts = []
            for inst in bb.instructions:
                si = inst.sync_info
                if si is not None and si.on_wait and len(si.on_wait) > max_waits:
                    waits = list(si.on_wait)
                    extra, keep = waits[:-max_waits], waits[-max_waits:]
                    for k, w in enumerate(extra):
                        nop = mybir.InstNoOp(name=f"{inst.name}_wsplit{k}")
                        nop.engine = inst.engine
                        nop.sync_info = mybir.SyncInfo(on_wait=[w], on_update=[])
                        new_insts.append(nop)
                    inst.sync_info = mybir.SyncInfo(
                        on_wait=keep, on_update=list(si.on_update)
                    )
                new_insts.append(inst)
            bb.instructions = new_insts


def build_nc(split_waits=True, bufs_e=4, bufs_raw=6, bufs_vraw=3, bufs_k=2, bufs_vx=2):
    nc = bass.Bass("TRN2", target_bir_lowering=False, debug=False)

    # Host-blocked layouts (see make_in_maps):
    #   QTB[qb*128+p, c*512+u] = Q[b, qb*512+u, c*128+p]
    #   KTB[sb*128+p, c*512+u] = K[b, h*2048 + sb*512+u, c*128+p]
    #   VTB[sb*128+p, u*1024 + c*128+q] = V[b, h*2048 + (4*sb+u)*128+q, c*128+p]
    #   WxB[p, c*128+k]        = Wx[c*128+p, k]
    #   MKB[p, j]              = (mask[b, 0, h*2048 + j*128+p] == 1)
    qt_d = nc.dram_tensor("QTB", [NQB * P, NDC * 512], F16, kind="ExternalInput").ap()
    kt_d = nc.dram_tensor("KTB", [NSB * P, NDC * 512], F16, kind="ExternalInput").ap()
    vt_d = nc.dram_tensor("VTB", [NSB * P, JPB * NDC * P], F16, kind="ExternalInput").ap()
    wq_d = nc.dram_tensor("WQB", [P, NDC * DK], F16, kind="ExternalInput").ap()
    wk_d = nc.dram_tensor("WKB", [P, NDC * DK], F16, kind="ExternalInput").ap()
    wv_d = nc.dram_tensor("WVB", [P, NDC * DV], F16, kind="ExternalInput").ap()
    mk_d = nc.dram_tensor("MKB", [P, NST], F32, kind="ExternalInput").ap()
    # numerator (cols 0:128) + denominator (col 128) per query
    o_d = nc.dram_tensor("O", [LQ, VW], F32, kind="ExternalOutput").ap()

    with tile.TileContext(nc) as tc:
        from contextlib import ExitStack

        with ExitStack() as ctx:
            # ---- SBUF pools ----
            wpool = ctx.enter_context(tc.tile_pool(name="w", bufs=1))
            per = ctx.enter_context(tc.tile_pool(name="per", bufs=1))
            kpool = ctx.enter_context(tc.tile_pool(name="kp", bufs=bufs_k))
            vxpool = ctx.enter_context(tc.tile_pool(name="vx", bufs=bufs_vx))
            epool = ctx.enter_context(tc.tile_pool(name="e", bufs=bufs_e))
            raw = ctx.enter_context(tc.tile_pool(name="raw", bufs=bufs_raw))
            vraw = ctx.enter_context(tc.tile_pool(name="vraw", bufs=bufs_vraw))
            # ---- PSUM pools (1 + 1 + 4 + 2 = 8 banks) ----
            pk = ctx.enter_context(tc.tile_pool(name="pk", bufs=1, space="PSUM"))
            pv = ctx.enter_context(tc.tile_pool(name="pv", bufs=1, space="PSUM"))
            ps = ctx.enter_context(tc.tile_pool(name="ps", bufs=2, space="PSUM"))
            pav = ctx.enter_context(tc.tile_pool(name="pav", bufs=2, space="PSUM"))

            # ---- load weights + mask (wk first: k-projection starts first) ----
            wq = wpool.tile([P, NDC * DK], F16)
            wk = wpool.tile([P, NDC * DK], F16)
            wv = wpool.tile([P, NDC * DV], F16)
            mkb = wpool.tile([P, NST], F32)
            for half in range(2):
                nc.sync.dma_start(
                    wk[:, half * 512 : (half + 1) * 512],
                    wk_d[:, half * 512 : (half + 1) * 512],
                )
            nc.sync.dma_start(wq[:], wq_d[:])
            nc.sync.dma_start(wv[:], wv_d[:])
            nc.sync.dma_start(mkb[:], mk_d[:])

            # ---- persistent state ----
            qT = per.tile([P, LQ], F16)                     # [d, q]
            acc = per.tile([P, NQB * NQT_PER_B * VW], F32)  # per q-tile [q, 129]

            def k_part(sb):
                kr = raw.tile([P, NDC * 512], F16, tag="kraw", name=f"kr{sb}")
                if sb == 0:
                    for c in range(NDC):
                        nc.sync.dma_start(
                            kr[:, c * 512 : (c + 1) * 512],
                            kt_d[0:P, c * 512 : (c + 1) * 512],
                        )
                else:
                    nc.sync.dma_start(kr[:], kt_d[sb * P : (sb + 1) * P, :])
                psk = pk.tile([P, 512], F32, tag="pproj", name=f"psk{sb}")
                for c in range(NDC):
                    nc.tensor.matmul(
                        psk[:],
                        wk[:, c * DK : (c + 1) * DK],
                        kr[:, c * 512 : (c + 1) * 512],
                        start=(c == 0),
                        stop=(c == NDC - 1),
                    )
                kTb = kpool.tile([P, 512], F16, tag="ktb", name=f"kTb{sb}")
                nc.vector.tensor_copy(kTb[:], psk[:])
                return kTb

            def v_part(sb):
                vr = vraw.tile([P, JPB * NDC * P], F16, tag="vraw", name=f"vr{sb}")
                if sb == 0:
                    w = NDC * P
                    for u in range(JPB):
                        nc.sync.dma_start(
                            vr[:, u * w : (u + 1) * w],
                            vt_d[0:P, u * w : (u + 1) * w],
                        )
                else:
                    nc.sync.dma_start(vr[:], vt_d[sb * P : (sb + 1) * P, :])
                vext = vxpool.tile([P, JPB * VW], F16, tag="vext", name=f"vext{sb}")
                for u in range(JPB):
                    j = sb * JPB + u
                    psv = pv.tile([P, DV], F32, tag="psv", name=f"psv{sb}_{u}")
                    for c in range(NDC):
                        nc.tensor.matmul(
                            psv[:],
                            vr[:, u * NDC * P + c * P : u * NDC * P + (c + 1) * P],
                            wv[:, c * DV : (c + 1) * DV],
                            start=(c == 0),
                            stop=(c == NDC - 1),
                        )
                    nc.vector.tensor_scalar_mul(
                        vext[:, u * VW : u * VW + DV], psv[:], mkb[:, j : j + 1]
                    )
                    nc.vector.tensor_copy(
                        vext[:, u * VW + DV : u * VW + VW], mkb[:, j : j + 1]
                    )
                return vext

            def qproj(qb):
                qr = raw.tile([P, NDC * 512], F16, tag="kraw", name=f"qr{qb}")
                if qb == 0:
                    for c in range(NDC):
                        nc.sync.dma_start(
                            qr[:, c * 512 : (c + 1) * 512],
                            qt_d[0:P, c * 512 : (c + 1) * 512],
                        )
                else:
                    nc.sync.dma_start(qr[:], qt_d[qb * P : (qb + 1) * P, :])
                psq = pk.tile([P, 512], F32, tag="pproj", name=f"psq{qb}")
                for c in range(NDC):
                    nc.tensor.matmul(
                        psq[:],
                        wq[:, c * DK : (c + 1) * DK],
                        qr[:, c * 512 : (c + 1) * 512],
                        start=(c == 0),
                        stop=(c == NDC - 1),
                    )
                nc.vector.tensor_copy(qT[:, qb * 512 : (qb + 1) * 512], psq[:])

            def scores_exp(sb, qb, kTb):
                ets = []
                for u2 in range(JPB // 2):
                    pss = ps.tile([P, 1024], F32, tag="pss", name=f"pss{sb}_{qb}_{u2}")
                    for v2 in range(2):
                        u = u2 * 2 + v2
                        nc.tensor.matmul(
                            pss[:, v2 * 512 : (v2 + 1) * 512],
                            kTb[:, u * P : (u + 1) * P],
                            qT[:, qb * 512 : (qb + 1) * 512],
                            start=True,
                            stop=True,
                        )
                    et = epool.tile([P, 1024], F16, tag="e", name=f"et{sb}_{qb}_{u2}")
                    nc.scalar.activation(
                        et[:], pss[:], mybir.ActivationFunctionType.Exp, scale=SCALE
                    )
                    ets.append(et)
                return ets

            def av_acc(sb, qb, ets, vext):
                # two q-tiles share one psum bank / one accumulation group
                # (258 f32 cols < 512); one DVE drain per pair
                for tp in range(NQT_PER_B // 2):
                    avp = pav.tile(
                        [P, 2 * VW], F32, tag="av", name=f"av{sb}_{qb}_{tp}"
                    )
                    nmm = 2 * JPB
                    for i in range(nmm):
                        half, u = divmod(i, JPB)
                        t = tp * 2 + half
                        et = ets[u // 2]
                        off = (u % 2) * 512
                        nc.tensor.matmul(
                            avp[:, half * VW : (half + 1) * VW],
                            et[:, off + t * P : off + (t + 1) * P],
                            vext[:, u * VW : (u + 1) * VW],
                            start=(i == 0),
                            stop=(i == nmm - 1),
                            skip_group_check=True,
                        )
                    g = (qb * NQT_PER_B + tp * 2) * VW
                    if sb == 0:
                        nc.vector.tensor_copy(acc[:, g : g + 2 * VW], avp[:])
                    else:
                        nc.vector.tensor_add(
                            acc[:, g : g + 2 * VW], acc[:, g : g + 2 * VW], avp[:]
                        )

            def finalize(qb):
                # ship numerator+denominator; host combines the key halves
                g0 = qb * NQT_PER_B * VW
                dst = o_d[qb * 512 : (qb + 1) * 512, :].rearrange(
                    "(t p) d -> p t d", p=P
                )
                src = acc[:, g0 : g0 + NQT_PER_B * VW].rearrange(
                    "p (t d) -> p t d", d=VW
                )
                nc.sync.dma_start(dst, src)

            # ---- software-pipelined main loop: AV lags scores by one unit,
            # next block's K/V projection slides in before the last AV ----
            kTbs = {0: k_part(0)}
            vexts = {}
            pending = None

            def drain_pending():
                psb, pqb, pets = pending
                av_acc(psb, pqb, pets, vexts[psb])
                if psb == NSB - 1:
                    finalize(pqb)

            for sb in range(NSB):
                for qb in range(NQB):
                    if sb == 0:
                        qproj(qb)
                    ets = scores_exp(sb, qb, kTbs[sb])
                    if sb == 0 and qb == 0:
                        vexts[0] = v_part(0)
                    if pending is not None:
                        drain_pending()
                    pending = (sb, qb, ets)
                    if qb == NQB - 1 and sb + 1 < NSB:
                        kTbs[sb + 1] = k_part(sb + 1)
                        vexts[sb + 1] = v_part(sb + 1)
            drain_pending()

    if split_waits:
        _split_multi_waits(nc)
    return nc


_NC = None


def _get_nc():
    global _NC
    if _NC is None:
        _NC = build_nc()
    return _NC


def _block2(x, rows):
    """x [S, DM] -> blocked [S//rows * P, NDC*rows]:
    out[blk*P + p, c*rows + u] = x[blk*rows + u, c*P + p]"""
    S = x.shape[0]
    nblk = S // rows
    r = x.reshape(nblk, rows, NDC, P)
    return np.ascontiguousarray(r.transpose(0, 3, 2, 1)).reshape(nblk * P, NDC * rows)


def make_in_maps(Q, K, V, mask, WQ, WK, WV):
    f16 = np.float16
    Q = np.asarray(Q, dtype=np.float32)
    K = np.asarray(K, dtype=np.float32)
    V = np.asarray(V, dtype=np.float32)
    mask = np.asarray(mask)

    def wblock(W):
        w = np.asarray(W, dtype=np.float32).astype(f16)
        return np.ascontiguousarray(w.reshape(NDC, P, DK).transpose(1, 0, 2)).reshape(
            P, NDC * DK
        )

    wqb, wkb, wvb = wblock(WQ), wblock(WK), wblock(WV)

    in_maps = []
    for c in range(N_CORES):
        b, h = c // 2, c % 2
        if h == 0:
            qtb_b = _block2(Q[b].astype(f16), 512)  # shared by both halves
        ksl = slice(h * LK, (h + 1) * LK)
        ktb = _block2(K[b, ksl].astype(f16), 512)
        vtb = _block2(V[b, ksl].astype(f16), P)  # [16*128, 1024]
        vtb = np.ascontiguousarray(
            vtb.reshape(NSB, JPB, P, NDC * P).transpose(0, 2, 1, 3)
        ).reshape(NSB * P, JPB * NDC * P)
        mkb = np.ascontiguousarray(
            (mask[b, 0, ksl] == 1).astype(np.float32).reshape(NST, P).T
        )
        in_maps.append(
            {
                "QTB": qtb_b,
                "KTB": ktb,
                "VTB": vtb,
                "WQB": wqb,
                "WKB": wkb,
                "WVB": wvb,
                "MKB": mkb,
            }
        )
    return in_maps


def assemble(results):
    out = np.empty((B, L, DV), dtype=np.float32)
    for b in range(B):
        a0 = results[2 * b]["O"]
        a1 = results[2 * b + 1]["O"]
        num = a0[:, :DV] + a1[:, :DV]
        den = a0[:, DV:] + a1[:, DV:]
        out[b] = num / den
    return out


def kernel(Q, K, V, mask, WQ, WK, WV):
    in_maps = make_in_maps(Q, K, V, mask, WQ, WK, WV)
    try:
        res = run_bass_kernel_spmd(_get_nc(), in_maps, core_ids=list(range(N_CORES)))
    except Exception:
        # transient device faults (e.g. a wedged core from a prior run)
        # usually clear on retry
        import time as _time

        _time.sleep(2.0)
        res = run_bass_kernel_spmd(_get_nc(), in_maps, core_ids=list(range(N_CORES)))
    return assemble(res.results)



# revision 2
# speedup vs baseline: 11.8265x; 11.8265x over previous
"""Bass/Trainium2 kernel v2 for nn_Attention_6983616824195.

Single-head attention with Dense projections:
    q = Q @ WQ ; k = K @ WK ; v = V @ WV        (B, L, 128)
    S = q @ k^T ; S = where(mask==1, S, -inf) ; S /= sqrt(128)
    out = softmax(S, axis=-1) @ v               (B, L, 128)

Shapes: B=4, L=4096, DM=1024, DK=DV=128, mask [B, 1, L] (key mask).

Sharding: 8 cores = (batch b, KEY-half h). Core c = (b=c//2, h=c%2)
computes ALL queries of batch b against keys [h*2048, (h+1)*2048).
Each core returns the unnormalized softmax numerator plus denominator
for its key half in fp16; the host adds the two halves and divides.

v2 structure (vs v1): query-block-outer loop with K/V projected once
up front and kept SBUF-resident; the AV matmuls accumulate all 16
key-tiles of a query block directly in an open PSUM accumulation
group (address-based has_written semantics), eliminating the per-
key-block DVE adds of v1. Output is fp16 (halves output DMA bytes).

Per-core dataflow (all matmuls contract over the SBUF partition dim):
  Phase 1:
    kT_all[d, s]   = sum_c WK[c]^T·KTB[c]   for all 2048 keys
    qT(0)          = first query block projected
    vext_all[s, j*129+{0:128}] = (V proj)·mask[s];  col 128 = mask[s]
  Phase 2, per query block qb (512 queries, 8 blocks):
    qT(qb+1) projected (pipelined)
    per j2 in 0..7 (two key-tiles each):
      S^T[s, q] = kT^T·qT  (two matmuls into one [128,1024] psum)
      e = exp(S^T/sqrt(128)) -> fp16 tile   (one ScalarE op)
      AV: 8 matmuls accumulate e^T·vext into 2 open psum banks
          (2 q-tiles per bank, cols 0:129/129:258; group spans all j2)
    drain banks (DVE f32->f16) and DMA out.
Masking is exact: masked keys get weight mask[s]=0 in both numerator
and denominator, identical to where(mask==1, S, -inf) softmax.
"""

import numpy as np

import jax

try:  # persistent compile cache: repeat calls skip the walrus compile
    jax.config.update("jax_compilation_cache_dir", "/tmp/jaxcache")
    jax.config.update("jax_persistent_cache_min_compile_time_secs", 1.0)
    jax.config.update("jax_persistent_cache_min_entry_size_bytes", 0)
except Exception:
    pass

import concourse.bass as bass
import concourse.tile as tile
import concourse.mybir as mybir
from concourse.bass_utils import run_bass_kernel_spmd

B, L, DM = 4, 4096, 1024
DK = DV = 128
N_CORES = 8
LQ = L                 # queries per core (all 4096 of the batch)
LK = L // 2            # keys per core (2048)
P = 128
NDC = DM // P          # dm chunks (8)
NQB = LQ // 512        # q blocks of 512 (8)
NQT_PER_B = 512 // P   # q tiles per block (4)
NST = LK // P          # s tiles per core (16)
NJ2 = NST // 2         # score steps per q block (8)
VW = DV + 1            # v-ext width (129): 128 dv cols + ones column
SCALE = 1.0 / float(np.sqrt(DK))

F32 = mybir.dt.float32
F16 = mybir.dt.float16


def _split_multi_waits(nc, max_waits=1):
    """This walrus build encodes at most one sync-wait per instruction;
    move surplus waits onto preceding NoOps on the same engine."""
    for f in nc.m.functions:
        for bb in f.blocks:
            new_insts = []
            for inst in bb.instructions:
                si = inst.sync_info
                if si is not None and si.on_wait and len(si.on_wait) > max_waits:
                    waits = list(si.on_wait)
                    extra, keep = waits[:-max_waits], waits[-max_waits:]
                    for k, w in enumerate(extra):
                        nop = mybir.InstNoOp(name=f"{inst.name}_wsplit{k}")
                        nop.engine = inst.engine
                        nop.sync_info = mybir.SyncInfo(on_wait=[w], on_update=[])
                        new_insts.append(nop)
                    inst.sync_info = mybir.SyncInfo(
                        on_wait=keep, on_update=list(si.on_update)
                    )
                new_insts.append(inst)
            bb.instructions = new_insts


def build_nc(split_waits=True, bufs_raw=6, bufs_e=4):
    nc = bass.Bass("TRN2", target_bir_lowering=False, debug=False)

    # ONE packed fp16 input buffer (single PJRT operand per exec):
    #   rows 0:128      weights stripe: cols 0:1024 WQB, 1024:2048 WKB,
    #                   2048:3072 WVB, 3072:3088 MKB (0.0/1.0 in fp16)
    #   rows 128:1152   QTB[qb*128+p, c*512+u] = Q[b, qb*512+u, c*128+p]
    #   rows 1152:1664  KTB[sb*128+p, c*512+u] = K[b, h*2048+sb*512+u, c*128+p]
    #   rows 1664:2176  VTB[sb*128+p, u*1024+c*128+q]
    #                     = V[b, h*2048 + (4*sb+u)*128+q, c*128+p]
    #   WxB[p, c*128+k] = Wx[c*128+p, k]
    #   MKB[p, j]       = (mask[b, 0, h*2048 + j*128+p] == 1)
    in_d = nc.dram_tensor("IN", [2176, NDC * 512], F16, kind="ExternalInput").ap()
    wq_d = in_d[0:P, 0:1024]
    wk_d = in_d[0:P, 1024:2048]
    wv_d = in_d[0:P, 2048:3072]
    mk_d = in_d[0:P, 3072 : 3072 + NST]
    qt_d = in_d[P : P + NQB * P, :]
    kt_d = in_d[1152 : 1152 + 4 * P, :]
    vt_d = in_d[1664 : 1664 + 4 * P, :]
    # numerator (cols 0:128) + denominator (col 128) per query, fp16
    o_d = nc.dram_tensor("O", [LQ, VW], F16, kind="ExternalOutput").ap()

    with tile.TileContext(nc) as tc:
        from contextlib import ExitStack

        with ExitStack() as ctx:
            # ---- SBUF pools ----
            wpool = ctx.enter_context(tc.tile_pool(name="w", bufs=1))
            per = ctx.enter_context(tc.tile_pool(name="per", bufs=1))
            raw = ctx.enter_context(tc.tile_pool(name="raw", bufs=bufs_raw))
            qpool = ctx.enter_context(tc.tile_pool(name="qp", bufs=2))
            epool = ctx.enter_context(tc.tile_pool(name="e", bufs=bufs_e))
            opool = ctx.enter_context(tc.tile_pool(name="o", bufs=2))
            # ---- PSUM pools (2 + 4 + 2 = 8 banks) ----
            pp = ctx.enter_context(tc.tile_pool(name="pp", bufs=2, space="PSUM"))
            ps = ctx.enter_context(tc.tile_pool(name="ps", bufs=2, space="PSUM"))
            pav = ctx.enter_context(tc.tile_pool(name="pav", bufs=1, space="PSUM"))

            # ---- load weights + mask (wk first: k-projection starts first) ----
            wq = wpool.tile([P, NDC * DK], F16)
            wk = wpool.tile([P, NDC * DK], F16)
            wv = wpool.tile([P, NDC * DV], F16)
            mkb16 = wpool.tile([P, NST], F16)
            mkb = wpool.tile([P, NST], F32)
            nc.sync.dma_start(wk[:], wk_d[:])

            # ---- persistent state ----
            kT_all = per.tile([P, LK], F16)          # [d, s] all keys
            vext_all = per.tile([P, NST * VW], F16)  # [s, j*129 + (dv|mask)]

            def k_part(sb):
                kr = raw.tile([P, NDC * 512], F16, tag="raw", name=f"kr{sb}")
                if sb == 0:
                    for half in range(2):
                        nc.sync.dma_start(
                            kr[:, half * 2048 : (half + 1) * 2048],
                            kt_d[0:P, half * 2048 : (half + 1) * 2048],
                        )
                else:
                    nc.sync.dma_start(kr[:], kt_d[sb * P : (sb + 1) * P, :])
                psk = pp.tile([P, 512], F32, tag="pproj", name=f"psk{sb}")
                for c in range(NDC):
                    nc.tensor.matmul(
                        psk[:],
                        wk[:, c * DK : (c + 1) * DK],
                        kr[:, c * 512 : (c + 1) * 512],
                        start=(c == 0),
                        stop=(c == NDC - 1),
                    )
                nc.vector.tensor_copy(kT_all[:, sb * 512 : (sb + 1) * 512], psk[:])

            def v_part(sb):
                vr = raw.tile([P, 4 * NDC * P], F16, tag="raw", name=f"vr{sb}")
                nc.sync.dma_start(vr[:], vt_d[sb * P : (sb + 1) * P, :])
                for u in range(4):
                    j = sb * 4 + u
                    psv = pp.tile([P, DV], F32, tag="pproj", name=f"psv{sb}_{u}")
                    for c in range(NDC):
                        nc.tensor.matmul(
                            psv[:],
                            vr[:, u * NDC * P + c * P : u * NDC * P + (c + 1) * P],
                            wv[:, c * DV : (c + 1) * DV],
                            start=(c == 0),
                            stop=(c == NDC - 1),
                        )
                    nc.vector.tensor_scalar_mul(
                        vext_all[:, j * VW : j * VW + DV], psv[:], mkb[:, j : j + 1]
                    )
                    nc.gpsimd.tensor_copy(
                        vext_all[:, j * VW + DV : j * VW + VW], mkb[:, j : j + 1]
                    )

            def qproj(qb):
                qr = raw.tile([P, NDC * 512], F16, tag="raw", name=f"qr{qb}")
                if qb == 0:
                    for half in range(2):
                        nc.sync.dma_start(
                            qr[:, half * 2048 : (half + 1) * 2048],
                            qt_d[0:P, half * 2048 : (half + 1) * 2048],
                        )
                else:
                    nc.sync.dma_start(qr[:], qt_d[qb * P : (qb + 1) * P, :])
                psq = pp.tile([P, 512], F32, tag="pproj", name=f"psq{qb}")
                for c in range(NDC):
                    nc.tensor.matmul(
                        psq[:],
                        wq[:, c * DK : (c + 1) * DK],
                        qr[:, c * 512 : (c + 1) * 512],
                        start=(c == 0),
                        stop=(c == NDC - 1),
                    )
                qTb = qpool.tile([P, 512], F16, tag="qtb", name=f"qTb{qb}")
                nc.vector.tensor_copy(qTb[:], psq[:])
                return qTb

            def scores_exp(qb, j2, qTb):
                pss = ps.tile([P, 1024], F32, tag="pss", name=f"pss{qb}_{j2}")
                for v2 in range(2):
                    j = 2 * j2 + v2
                    nc.tensor.matmul(
                        pss[:, v2 * 512 : (v2 + 1) * 512],
                        kT_all[:, j * P : (j + 1) * P],
                        qTb[:],
                        start=True,
                        stop=True,
                    )
                et = epool.tile([P, 1024], F16, tag="e", name=f"et{qb}_{j2}")
                nc.scalar.activation(
                    et[:], pss[:], mybir.ActivationFunctionType.Exp, scale=SCALE
                )
                return et

            def av_step(qb, j2, et, avps):
                # 2 q-tiles share each bank; group stays open across all j2
                for tp in range(2):
                    for i in range(4):
                        v2, half = divmod(i, 2)
                        t = tp * 2 + half
                        j = 2 * j2 + v2
                        nc.tensor.matmul(
                            avps[tp][:, half * VW : (half + 1) * VW],
                            et[:, v2 * 512 + t * P : v2 * 512 + (t + 1) * P],
                            vext_all[:, j * VW : (j + 1) * VW],
                            start=(j2 == 0 and i == 0),
                            stop=(j2 == NJ2 - 1 and i == 3),
                            skip_group_check=True,
                        )

            def finalize(qb, avps):
                ot = opool.tile([P, NQT_PER_B * VW], F16, tag="ot", name=f"ot{qb}")
                for tp in range(2):
                    nc.vector.tensor_copy(
                        ot[:, tp * 2 * VW : (tp + 1) * 2 * VW], avps[tp][:]
                    )
                dst = o_d[qb * 512 : (qb + 1) * 512, :].rearrange(
                    "(t p) d -> p t d", p=P
                )
                src = ot[:].rearrange("p (t d) -> p t d", d=VW)
                nc.sync.dma_start(dst, src)

            # ---- phase 1: K projection, first q block, V projection ----
            k_part(0)
            nc.sync.dma_start(wq[:], wq_d[:])
            nc.sync.dma_start(wv[:], wv_d[:])
            nc.sync.dma_start(mkb16[:], mk_d[:])
            nc.vector.tensor_copy(mkb[:], mkb16[:])
            for sb in range(1, 4):
                k_part(sb)
            qTbs = {0: qproj(0)}
            for sb in range(4):
                v_part(sb)

            # ---- phase 2: main loop, AV lags scores/exp by one step ----
            pending = None  # (qb, j2, et, avps)
            for qb in range(NQB):
                avps = [
                    pav.tile([P, 2 * VW], F32, tag=f"av{tp}", name=f"av{qb}_{tp}")
                    for tp in range(2)
                ]
                qTb = qTbs[qb]
                for j2 in range(NJ2):
                    et = scores_exp(qb, j2, qTb)
                    if pending is not None:
                        p_qb, p_j2, p_et, p_avps = pending
                        av_step(p_qb, p_j2, p_et, p_avps)
                        if p_j2 == NJ2 - 1:
                            finalize(p_qb, p_avps)
                    pending = (qb, j2, et, avps)
                    if j2 == NJ2 - 1 and qb + 1 < NQB:
                        qTbs[qb + 1] = qproj(qb + 1)
            p_qb, p_j2, p_et, p_avps = pending
            av_step(p_qb, p_j2, p_et, p_avps)
            finalize(p_qb, p_avps)

    if split_waits:
        _split_multi_waits(nc)
    return nc


_NC = None


def _get_nc():
    global _NC
    if _NC is None:
        _NC = build_nc()
    return _NC


def _block2(x, rows):
    """x [S, DM] -> blocked [S//rows * P, NDC*rows]:
    out[blk*P + p, c*rows + u] = x[blk*rows + u, c*P + p]"""
    S = x.shape[0]
    nblk = S // rows
    r = x.reshape(nblk, rows, NDC, P)
    return np.ascontiguousarray(r.transpose(0, 3, 2, 1)).reshape(nblk * P, NDC * rows)


def make_in_maps(Q, K, V, mask, WQ, WK, WV):
    f16 = np.float16
    Q = np.asarray(Q, dtype=np.float32)
    K = np.asarray(K, dtype=np.float32)
    V = np.asarray(V, dtype=np.float32)
    mask = np.asarray(mask)

    def wblock(W):
        w = np.asarray(W, dtype=np.float32).astype(f16)
        return np.ascontiguousarray(w.reshape(NDC, P, DK).transpose(1, 0, 2)).reshape(
            P, NDC * DK
        )

    wqb, wkb, wvb = wblock(WQ), wblock(WK), wblock(WV)

    in_maps = []
    for c in range(N_CORES):
        b, h = c // 2, c % 2
        if h == 0:
            qtb_b = _block2(Q[b].astype(f16), 512)  # shared by both halves
        ksl = slice(h * LK, (h + 1) * LK)
        ktb = _block2(K[b, ksl].astype(f16), 512)
        vtb = _block2(V[b, ksl].astype(f16), P)  # [16*128, 1024]
        vtb = np.ascontiguousarray(
            vtb.reshape(4, 4, P, NDC * P).transpose(0, 2, 1, 3)
        ).reshape(4 * P, 4 * NDC * P)
        mkb = np.ascontiguousarray(
            (mask[b, 0, ksl] == 1).astype(f16).reshape(NST, P).T
        )
        packed = np.zeros((2176, NDC * 512), dtype=f16)
        packed[0:P, 0:1024] = wqb
        packed[0:P, 1024:2048] = wkb
        packed[0:P, 2048:3072] = wvb
        packed[0:P, 3072 : 3072 + NST] = mkb
        packed[P : P + NQB * P, :] = qtb_b
        packed[1152 : 1152 + 4 * P, :] = ktb
        packed[1664 : 1664 + 4 * P, :] = vtb
        in_maps.append({"IN": packed})
    return in_maps


def assemble(results):
    out = np.empty((B, L, DV), dtype=np.float32)
    for b in range(B):
        a0 = np.asarray(results[2 * b]["O"], dtype=np.float32)
        a1 = np.asarray(results[2 * b + 1]["O"], dtype=np.float32)
        num = a0[:, :DV] + a1[:, :DV]
        den = a0[:, DV:] + a1[:, DV:]
        out[b] = num / den
    return out


def kernel(Q, K, V, mask, WQ, WK, WV):
    in_maps = make_in_maps(Q, K, V, mask, WQ, WK, WV)
    try:
        res = run_bass_kernel_spmd(_get_nc(), in_maps, core_ids=list(range(N_CORES)))
    except Exception:
        # transient device faults (e.g. a wedged core from a prior run)
        # usually clear on retry
        import time as _time

        _time.sleep(2.0)
        res = run_bass_kernel_spmd(_get_nc(), in_maps, core_ids=list(range(N_CORES)))
    return assemble(res.results)


# revision 3
# speedup vs baseline: 205.1807x; 17.3493x over previous
"""Bass/Trainium2 kernel v2 for nn_Attention_6983616824195.

Single-head attention with Dense projections:
    q = Q @ WQ ; k = K @ WK ; v = V @ WV        (B, L, 128)
    S = q @ k^T ; S = where(mask==1, S, -inf) ; S /= sqrt(128)
    out = softmax(S, axis=-1) @ v               (B, L, 128)

Shapes: B=4, L=4096, DM=1024, DK=DV=128, mask [B, 1, L] (key mask).

Sharding: 8 cores = (batch b, KEY-half h). Core c = (b=c//2, h=c%2)
computes ALL queries of batch b against keys [h*2048, (h+1)*2048).
Each core returns the unnormalized softmax numerator plus denominator
for its key half in fp16; the host adds the two halves and divides.

v2 structure (vs v1): query-block-outer loop with K/V projected once
up front and kept SBUF-resident; the AV matmuls accumulate all 16
key-tiles of a query block directly in an open PSUM accumulation
group (address-based has_written semantics), eliminating the per-
key-block DVE adds of v1. Output is fp16 (halves output DMA bytes).

Per-core dataflow (all matmuls contract over the SBUF partition dim):
  Phase 1:
    kT_all[d, s]   = sum_c WK[c]^T·KTB[c]   for all 2048 keys
    qT(0)          = first query block projected
    vext_all[s, j*129+{0:128}] = (V proj)·mask[s];  col 128 = mask[s]
  Phase 2, per query block qb (512 queries, 8 blocks):
    qT(qb+1) projected (pipelined)
    per j2 in 0..7 (two key-tiles each):
      S^T[s, q] = kT^T·qT  (two matmuls into one [128,1024] psum)
      e = exp(S^T/sqrt(128)) -> fp16 tile   (one ScalarE op)
      AV: 8 matmuls accumulate e^T·vext into 2 open psum banks
          (2 q-tiles per bank, cols 0:129/129:258; group spans all j2)
    drain banks (DVE f32->f16) and DMA out.
Masking is exact: masked keys get weight mask[s]=0 in both numerator
and denominator, identical to where(mask==1, S, -inf) softmax.
"""

import numpy as np

import jax

try:  # persistent compile cache: repeat calls skip the walrus compile
    jax.config.update("jax_compilation_cache_dir", "/tmp/jaxcache")
    jax.config.update("jax_persistent_cache_min_compile_time_secs", 1.0)
    jax.config.update("jax_persistent_cache_min_entry_size_bytes", 0)
except Exception:
    pass

import concourse.bass as bass
import concourse.tile as tile
import concourse.mybir as mybir
from concourse.bass_utils import run_bass_kernel_spmd

B, L, DM = 4, 4096, 1024
DK = DV = 128
N_CORES = 8
LQ = L                 # queries per core (all 4096 of the batch)
LK = L // 2            # keys per core (2048)
P = 128
NDC = DM // P          # dm chunks (8)
NQB = LQ // 512        # q blocks of 512 (8)
NQT_PER_B = 512 // P   # q tiles per block (4)
NST = LK // P          # s tiles per core (16)
NJ2 = NST // 2         # score steps per q block (8)
VW = DV + 1            # v-ext width (129): 128 dv cols + ones column
SCALE = 1.0 / float(np.sqrt(DK))

F32 = mybir.dt.float32
F16 = mybir.dt.float16


def _split_multi_waits(nc, max_waits=1):
    """This walrus build encodes at most one sync-wait per instruction;
    move surplus waits onto preceding NoOps on the same engine."""
    for f in nc.m.functions:
        for bb in f.blocks:
            new_insts = []
            for inst in bb.instructions:
                si = inst.sync_info
                if si is not None and si.on_wait and len(si.on_wait) > max_waits:
                    waits = list(si.on_wait)
                    extra, keep = waits[:-max_waits], waits[-max_waits:]
                    for k, w in enumerate(extra):
                        nop = mybir.InstNoOp(name=f"{inst.name}_wsplit{k}")
                        nop.engine = inst.engine
                        nop.sync_info = mybir.SyncInfo(on_wait=[w], on_update=[])
                        new_insts.append(nop)
                    inst.sync_info = mybir.SyncInfo(
                        on_wait=keep, on_update=list(si.on_update)
                    )
                new_insts.append(inst)
            bb.instructions = new_insts


def build_nc(split_waits=True, bufs_raw=6, bufs_e=4):
    nc = bass.Bass("TRN2", target_bir_lowering=False, debug=False)

    # ONE packed fp16 input buffer (single PJRT operand per exec):
    #   rows 0:128      weights stripe: cols 0:1024 WQB, 1024:2048 WKB,
    #                   2048:3072 WVB, 3072:3088 MKB (0.0/1.0 in fp16)
    #   rows 128:1152   QTB[qb*128+p, c*512+u] = Q[b, qb*512+u, c*128+p]
    #   rows 1152:1664  KTB[sb*128+p, c*512+u] = K[b, h*2048+sb*512+u, c*128+p]
    #   rows 1664:2176  VTB[sb*128+p, u*1024+c*128+q]
    #                     = V[b, h*2048 + (4*sb+u)*128+q, c*128+p]
    #   WxB[p, c*128+k] = Wx[c*128+p, k]
    #   MKB[p, j]       = (mask[b, 0, h*2048 + j*128+p] == 1)
    in_d = nc.dram_tensor("IN", [2176, NDC * 512], F16, kind="ExternalInput").ap()
    wq_d = in_d[0:P, 0:1024]
    wk_d = in_d[0:P, 1024:2048]
    wv_d = in_d[0:P, 2048:3072]
    mk_d = in_d[0:P, 3072 : 3072 + NST]
    qt_d = in_d[P : P + NQB * P, :]
    kt_d = in_d[1152 : 1152 + 4 * P, :]
    vt_d = in_d[1664 : 1664 + 4 * P, :]
    # numerator (cols 0:128) + denominator (col 128) per query, fp16
    o_d = nc.dram_tensor("O", [LQ, VW], F16, kind="ExternalOutput").ap()

    with tile.TileContext(nc) as tc:
        from contextlib import ExitStack

        with ExitStack() as ctx:
            # ---- SBUF pools ----
            wpool = ctx.enter_context(tc.tile_pool(name="w", bufs=1))
            per = ctx.enter_context(tc.tile_pool(name="per", bufs=1))
            raw = ctx.enter_context(tc.tile_pool(name="raw", bufs=bufs_raw))
            qpool = ctx.enter_context(tc.tile_pool(name="qp", bufs=2))
            epool = ctx.enter_context(tc.tile_pool(name="e", bufs=bufs_e))
            opool = ctx.enter_context(tc.tile_pool(name="o", bufs=2))
            # ---- PSUM pools (2 + 4 + 2 = 8 banks) ----
            pp = ctx.enter_context(tc.tile_pool(name="pp", bufs=2, space="PSUM"))
            ps = ctx.enter_context(tc.tile_pool(name="ps", bufs=2, space="PSUM"))
            pav = ctx.enter_context(tc.tile_pool(name="pav", bufs=1, space="PSUM"))

            # ---- load weights + mask (wk first: k-projection starts first) ----
            wq = wpool.tile([P, NDC * DK], F16)
            wk = wpool.tile([P, NDC * DK], F16)
            wv = wpool.tile([P, NDC * DV], F16)
            mkb16 = wpool.tile([P, NST], F16)
            mkb = wpool.tile([P, NST], F32)
            nc.sync.dma_start(wk[:], wk_d[:])

            # ---- persistent state ----
            kT_all = per.tile([P, LK], F16)          # [d, s] all keys
            vext_all = per.tile([P, NST * VW], F16)  # [s, j*129 + (dv|mask)]

            def dma_block(src, name, split=False):
                t = raw.tile([P, NDC * 512], F16, tag="raw", name=name)
                if split:
                    for half in range(2):
                        nc.sync.dma_start(
                            t[:, half * 2048 : (half + 1) * 2048],
                            src[:, half * 2048 : (half + 1) * 2048],
                        )
                else:
                    nc.sync.dma_start(t[:], src[:])
                return t

            def k_part(sb, kr):
                psk = pp.tile([P, 512], F32, tag="pproj", name=f"psk{sb}")
                for c in range(NDC):
                    nc.tensor.matmul(
                        psk[:],
                        wk[:, c * DK : (c + 1) * DK],
                        kr[:, c * 512 : (c + 1) * 512],
                        start=(c == 0),
                        stop=(c == NDC - 1),
                    )
                nc.vector.tensor_copy(kT_all[:, sb * 512 : (sb + 1) * 512], psk[:])

            def v_part(sb, vr, us):
                for u in us:
                    j = sb * 4 + u
                    psv = pp.tile([P, DV], F32, tag="pproj", name=f"psv{sb}_{u}")
                    for c in range(NDC):
                        nc.tensor.matmul(
                            psv[:],
                            vr[:, u * NDC * P + c * P : u * NDC * P + (c + 1) * P],
                            wv[:, c * DV : (c + 1) * DV],
                            start=(c == 0),
                            stop=(c == NDC - 1),
                        )
                    nc.vector.tensor_scalar_mul(
                        vext_all[:, j * VW : j * VW + DV], psv[:], mkb[:, j : j + 1]
                    )
                    nc.gpsimd.tensor_copy(
                        vext_all[:, j * VW + DV : j * VW + VW], mkb[:, j : j + 1]
                    )

            def qproj(qb, qr):
                psq = pp.tile([P, 512], F32, tag="pproj", name=f"psq{qb}")
                for c in range(NDC):
                    nc.tensor.matmul(
                        psq[:],
                        wq[:, c * DK : (c + 1) * DK],
                        qr[:, c * 512 : (c + 1) * 512],
                        start=(c == 0),
                        stop=(c == NDC - 1),
                    )
                qTb = qpool.tile([P, 512], F16, tag="qtb", name=f"qTb{qb}")
                nc.vector.tensor_copy(qTb[:], psq[:])
                return qTb

            def scores_exp(qb, j2, qTb):
                pss = ps.tile([P, 1024], F32, tag="pss", name=f"pss{qb}_{j2}")
                for v2 in range(2):
                    j = 2 * j2 + v2
                    nc.tensor.matmul(
                        pss[:, v2 * 512 : (v2 + 1) * 512],
                        kT_all[:, j * P : (j + 1) * P],
                        qTb[:],
                        start=True,
                        stop=True,
                    )
                et = epool.tile([P, 1024], F16, tag="e", name=f"et{qb}_{j2}")
                nc.scalar.activation(
                    et[:], pss[:], mybir.ActivationFunctionType.Exp, scale=SCALE
                )
                return et

            def av_step(qb, j2, et, avps):
                # 2 q-tiles share each bank; group stays open across all j2
                for tp in range(2):
                    for i in range(4):
                        v2, half = divmod(i, 2)
                        t = tp * 2 + half
                        j = 2 * j2 + v2
                        nc.tensor.matmul(
                            avps[tp][:, half * VW : (half + 1) * VW],
                            et[:, v2 * 512 + t * P : v2 * 512 + (t + 1) * P],
                            vext_all[:, j * VW : (j + 1) * VW],
                            start=(j2 == 0 and i == 0),
                            stop=(j2 == NJ2 - 1 and i == 3),
                            skip_group_check=True,
                        )

            def finalize(qb, avps):
                ot = opool.tile([P, NQT_PER_B * VW], F16, tag="ot", name=f"ot{qb}")
                for tp in range(2):
                    nc.vector.tensor_copy(
                        ot[:, tp * 2 * VW : (tp + 1) * 2 * VW], avps[tp][:]
                    )
                dst = o_d[qb * 512 : (qb + 1) * 512, :].rearrange(
                    "(t p) d -> p t d", p=P
                )
                src = ot[:].rearrange("p (t d) -> p t d", d=VW)
                nc.sync.dma_start(dst, src)

            # ---- prefetch DMAs in consumption order (one SP queue) ----
            krs, vrs = {}, {}
            krs[0] = dma_block(kt_d[0:P, :], "kr0", split=True)
            nc.sync.dma_start(wq[:], wq_d[:])
            qr0 = dma_block(qt_d[0:P, :], "qr0", split=True)
            nc.sync.dma_start(wv[:], wv_d[:])
            nc.sync.dma_start(mkb16[:], mk_d[:])
            vrs[0] = dma_block(vt_d[0:P, :], "vr0", split=True)
            for sb in range(1, 4):
                krs[sb] = dma_block(kt_d[sb * P : (sb + 1) * P, :], f"kr{sb}")
                vrs[sb] = dma_block(vt_d[sb * P : (sb + 1) * P, :], f"vr{sb}")
            nc.vector.tensor_copy(mkb[:], mkb16[:])

            # ---- pipelined main loop: K/V projection interleaved into the
            # first q block's steps; AV lags scores/exp by one step ----
            k_part(0, krs[0])
            qTbs = {0: qproj(0, qr0)}
            pending = None  # (qb, j2, et, avps)
            for qb in range(NQB):
                avps = [
                    pav.tile([P, 2 * VW], F32, tag=f"av{tp}", name=f"av{qb}_{tp}")
                    for tp in range(2)
                ]
                qTb = qTbs[qb]
                qr_next = None
                for j2 in range(NJ2):
                    et = scores_exp(qb, j2, qTb)
                    if qb == 0:
                        half = j2 % 2
                        v_part(j2 // 2, vrs[j2 // 2], (2 * half, 2 * half + 1))
                        if half == 1 and j2 < NJ2 - 1:
                            k_part((j2 + 1) // 2, krs[(j2 + 1) // 2])
                    if j2 == 0 and qb + 1 < NQB:
                        qr_next = dma_block(
                            qt_d[(qb + 1) * P : (qb + 2) * P, :], f"qr{qb + 1}"
                        )
                    if pending is not None:
                        p_qb, p_j2, p_et, p_avps = pending
                        av_step(p_qb, p_j2, p_et, p_avps)
                        if p_j2 == NJ2 - 1:
                            finalize(p_qb, p_avps)
                    pending = (qb, j2, et, avps)
                    if j2 == NJ2 - 1 and qb + 1 < NQB:
                        qTbs[qb + 1] = qproj(qb + 1, qr_next)
            p_qb, p_j2, p_et, p_avps = pending
            av_step(p_qb, p_j2, p_et, p_avps)
            finalize(p_qb, p_avps)

    if split_waits:
        _split_multi_waits(nc)
    return nc


_NC = None


def _get_nc():
    global _NC
    if _NC is None:
        _NC = build_nc()
    return _NC


def _block2(x, rows):
    """x [S, DM] -> blocked [S//rows * P, NDC*rows]:
    out[blk*P + p, c*rows + u] = x[blk*rows + u, c*P + p]"""
    S = x.shape[0]
    nblk = S // rows
    r = x.reshape(nblk, rows, NDC, P)
    return np.ascontiguousarray(r.transpose(0, 3, 2, 1)).reshape(nblk * P, NDC * rows)


def make_in_maps(Q, K, V, mask, WQ, WK, WV):
    f16 = np.float16
    Q = np.asarray(Q, dtype=np.float32)
    K = np.asarray(K, dtype=np.float32)
    V = np.asarray(V, dtype=np.float32)
    mask = np.asarray(mask)

    def wblock(W):
        w = np.asarray(W, dtype=np.float32).astype(f16)
        return np.ascontiguousarray(w.reshape(NDC, P, DK).transpose(1, 0, 2)).reshape(
            P, NDC * DK
        )

    wqb, wkb, wvb = wblock(WQ), wblock(WK), wblock(WV)

    in_maps = []
    for c in range(N_CORES):
        b, h = c // 2, c % 2
        if h == 0:
            qtb_b = _block2(Q[b].astype(f16), 512)  # shared by both halves
        ksl = slice(h * LK, (h + 1) * LK)
        ktb = _block2(K[b, ksl].astype(f16), 512)
        vtb = _block2(V[b, ksl].astype(f16), P)  # [16*128, 1024]
        vtb = np.ascontiguousarray(
            vtb.reshape(4, 4, P, NDC * P).transpose(0, 2, 1, 3)
        ).reshape(4 * P, 4 * NDC * P)
        mkb = np.ascontiguousarray(
            (mask[b, 0, ksl] == 1).astype(f16).reshape(NST, P).T
        )
        packed = np.zeros((2176, NDC * 512), dtype=f16)
        packed[0:P, 0:1024] = wqb
        packed[0:P, 1024:2048] = wkb
        packed[0:P, 2048:3072] = wvb
        packed[0:P, 3072 : 3072 + NST] = mkb
        packed[P : P + NQB * P, :] = qtb_b
        packed[1152 : 1152 + 4 * P, :] = ktb
        packed[1664 : 1664 + 4 * P, :] = vtb
        in_maps.append({"IN": packed})
    return in_maps


def assemble(results):
    out = np.empty((B, L, DV), dtype=np.float32)
    for b in range(B):
        a0 = np.asarray(results[2 * b]["O"], dtype=np.float32)
        a1 = np.asarray(results[2 * b + 1]["O"], dtype=np.float32)
        num = a0[:, :DV] + a1[:, :DV]
        den = a0[:, DV:] + a1[:, DV:]
        out[b] = num / den
    return out


def kernel(Q, K, V, mask, WQ, WK, WV):
    in_maps = make_in_maps(Q, K, V, mask, WQ, WK, WV)
    try:
        res = run_bass_kernel_spmd(_get_nc(), in_maps, core_ids=list(range(N_CORES)))
    except Exception:
        # transient device faults (e.g. a wedged core from a prior run)
        # usually clear on retry
        import time as _time

        _time.sleep(2.0)
        res = run_bass_kernel_spmd(_get_nc(), in_maps, core_ids=list(range(N_CORES)))
    return assemble(res.results)
